# revision 50
# baseline (speedup 1.0000x reference)
"""Trainium2 Bass kernel for a 6-layer geometry-constrained cross-attention decoder.

Sharding: pure data-parallel over batch B=8 -> one batch element per NeuronCore.
Per-core layouts are feature-major ("T" = transposed): activations live as
[feature, token] so every matmul streams with full 128-partition contraction
and biases are per-partition. Attention probabilities are computed unnormalized
(exp with no max-subtraction; scores are O(1) by construction), masked by a
multiplicative {0,1} bf16 mask on the vector engine, and normalized after the
AV matmul via a ones-column appended to V (row 32 of the AV accumulator is the
softmax denominator).

Everything bf16 except: the residual stream, layernorm statistics, biases and
PSUM accumulation (all fp32). Measured vs fp64 reference: ~1.8e-3 max rel err.
"""

import os
import sys

for _p in ("/opt/trn_rl_repo", "/root/.axon_site/_ro/trn_rl_repo"):
    if os.path.isdir(_p) and _p not in sys.path:
        sys.path.insert(0, _p)

import numpy as np
import ml_dtypes

import concourse.bass as bass
import concourse.tile as tile
from concourse import bacc
from concourse import mybir
from concourse import bass_utils

BF16 = ml_dtypes.bfloat16
F32 = np.float32

B, NQ, NK, E, H, F, L = 8, 300, 4096, 256, 8, 2048, 6
D = E // H
SCALE = D ** -0.5
PC = 128          # partitions
EC = E // PC      # 2 feature chunks
FT = F // PC      # 16 ffn chunks
KT_CA = NK // PC  # 32 cross-attention key tiles
NKC = NK // 512   # 8 key column chunks for k-projection
TOK_TILES = [(0, 100), (100, 100), (200, 100)]   # 300 tokens, uniform
G_EXP = 3         # k-tiles per exp batch

dt = mybir.dt
Alu = mybir.AluOpType
Act = mybir.ActivationFunctionType

# smalls column map (per-partition fp32 vectors, feature f = 128*c + p)
C_BQK = 0     # 4 cols: sa qk bias (q: 0:2, k: 2:4)
C_BO_SA = 4   # 2
C_BQ_CA = 6   # 2
C_BK_CA = 8   # 2
C_BO_CA = 10  # 2
C_B1 = 12     # 16
C_B2 = 28     # 2
C_LN = 30     # 12: ln1g ln1b ln2g ln2b ln3g ln3b (2 each)
NS = 42


def _bcmid(ap2d, c):
    """[P, N] AP -> [P, c, N] with the middle dim broadcast (step 0)."""
    return bass.AP(tensor=ap2d.tensor, offset=ap2d.offset,
                   ap=[list(ap2d.ap[0]), [0, c], list(ap2d.ap[-1])])


def _hd(ap):
    """reshape trailing 256-wide feature dim into [8, 32] head/dim"""
    return ap.rearrange("p (h d) -> p h d", d=D)


def build_nc(nlayers=L):
    nc = bacc.Bacc("TRN2", target_bir_lowering=False, debug=False)
    f32, bf, f32r = dt.float32, dt.bfloat16, dt.float32r

    def din(name, shape, d=bf):
        return nc.dram_tensor(name, shape, d, kind="ExternalInput").ap()

    d_tT = din("tT", [E, NQ], f32)
    d_memT = din("memT", [E, NK])
    d_maskT = din("maskT", [NK, NQ])
    d_wqk = din("w_sa_qk", [nlayers, E, 2 * E])
    d_wsv = din("w_sa_v", [nlayers, E, E])
    d_wso = din("w_sa_o", [nlayers, E, E])
    d_wcq = din("w_ca_q", [nlayers, E, E])
    d_wck = din("w_ca_k", [nlayers, E, E])
    d_wcv = din("w_ca_v", [nlayers, E, E])
    d_wco = din("w_ca_o", [nlayers, E, E])
    d_w1 = din("w_f1", [nlayers, E, F])
    d_w2 = din("w_f2", [nlayers, F, E])
    d_sm = din("smalls", [nlayers, PC, NS], f32)
    d_vrow = din("vrow", [nlayers, PC, 2, E], f32)
    d_fin = din("finals", [PC, 4], f32)
    d_out = nc.dram_tensor("outT", [E, NQ], f32, kind="ExternalOutput").ap()

    def r2(ap):  # [256, X] -> [128, 2, X]
        return ap.rearrange("(c p) o -> p c o", p=PC)

    with tile.TileContext(nc) as tc:
        with (
            tc.tile_pool(name="persist", bufs=1) as pst,
            tc.tile_pool(name="wts", bufs=2) as wp,
            tc.tile_pool(name="acts", bufs=2) as acts,
            tc.tile_pool(name="probs", bufs=6) as probs,
            tc.tile_pool(name="stats", bufs=2) as stp,
            tc.tile_pool(name="ps_sc", bufs=2, space="PSUM") as ps_sc,
            tc.tile_pool(name="ps_pp", bufs=2, space="PSUM") as ps_pp,
        ):
            # ---- persistent loads ----
            memT = pst.tile([PC, EC, NK], bf, tag="memT", name="memT_sb")
            nc.sync.dma_start(out=memT, in_=r2(d_memT))
            maskT = pst.tile([PC, KT_CA, NQ], bf, tag="maskT", name="maskT_sb")
            nc.sync.dma_start(out=maskT, in_=d_maskT.rearrange("(t p) q -> p t q", p=PC))
            ones = pst.tile([PC, PC], bf, tag="ones", name="ones_sb")
            nc.vector.memset(ones, 1.0)
            eps = pst.tile([PC, 1], f32, tag="eps", name="eps_sb")
            nc.vector.memset(eps, 1e-5)
            fin = pst.tile([PC, 4], f32, tag="fin", name="fin_sb")
            nc.sync.dma_start(out=fin, in_=d_fin)
            vsa = pst.tile([PC, len(TOK_TILES), H, 2 * D], bf, tag="vsa", name="vsa_sb")
            nc.vector.memset(vsa[:, :, :, D:2 * D], 1.0)
            vca = pst.tile([PC, KT_CA, H, 2 * D], bf, tag="vca", name="vca_sb")
            nc.vector.memset(vca[:, :, :, D:2 * D], 1.0)

            tT = acts.tile([PC, EC, NQ], f32, tag="tT", name="tT0")
            nc.sync.dma_start(out=tT, in_=r2(d_tT))
            tb = acts.tile([PC, EC, NQ], bf, tag="tb", name="tb0")
            nc.gpsimd.tensor_copy(out=tb, in_=tT)

            def layernorm(l, r, gcol, name):
                """r: [128, 2, 300] f32 (+ gets normalized) -> new (tT, tb)"""
                rb = acts.tile([PC, EC, NQ], bf, tag="rb", name=f"rb{name}", bufs=1)
                nc.vector.tensor_copy(out=rb, in_=r)
                tsq = acts.tile([PC, EC, NQ], bf, tag="tsq", name=f"tsq{name}", bufs=1)
                nc.vector.tensor_mul(out=tsq, in0=rb, in1=rb)
                s0 = ps_pp.tile([PC, NQ], f32, tag="pp", name=f"lns0{name}")
                s1 = ps_pp.tile([PC, NQ], f32, tag="pp", name=f"lns1{name}")
                for c in range(EC):
                    nc.tensor.matmul(out=s0, lhsT=ones,
                                     rhs=rb[:, c, :],
                                     start=(c == 0), stop=(c == EC - 1))
                for c in range(EC):
                    nc.tensor.matmul(out=s1, lhsT=ones,
                                     rhs=tsq[:, c, :],
                                     start=(c == 0), stop=(c == EC - 1))
                mean = stp.tile([PC, NQ], f32, tag="mean", name=f"mean{name}", bufs=1)
                nc.vector.tensor_scalar_mul(out=mean, in0=s0, scalar1=1.0 / E)
                c1 = acts.tile([PC, EC, NQ], f32, tag="c1", name=f"c1{name}", bufs=1)
                nc.vector.tensor_sub(out=c1, in0=r, in1=_bcmid(mean, EC))
                msq = stp.tile([PC, NQ], f32, tag="msq", name=f"msq{name}", bufs=1)
                nc.vector.tensor_mul(out=msq, in0=mean, in1=mean)
                var = stp.tile([PC, NQ], f32, tag="var", name=f"var{name}", bufs=1)
                nc.vector.scalar_tensor_tensor(out=var, in0=s1, scalar=1.0 / E,
                                               in1=msq, op0=Alu.mult, op1=Alu.subtract)
                sd = stp.tile([PC, NQ], f32, tag="sd", name=f"sd{name}", bufs=1)
                nc.scalar.activation(out=sd, in_=var, func=Act.Sqrt, bias=eps[:, 0:1])
                rstd = stp.tile([PC, NQ], f32, tag="rstd", name=f"rstd{name}", bufs=1)
                nc.vector.reciprocal(out=rstd, in_=sd)
                c2 = acts.tile([PC, EC, NQ], f32, tag="c2", name=f"c2{name}", bufs=1)
                nc.vector.tensor_mul(out=c2, in0=c1, in1=_bcmid(rstd, EC))
                t_new = acts.tile([PC, EC, NQ], f32, tag="tT", name=f"t{name}")
                tb_new = acts.tile([PC, EC, NQ], bf, tag="tb", name=f"tb{name}")
                if gcol is None:
                    g, b = fin[:, 0:2], fin[:, 2:4]
                else:
                    g = sm[:, gcol:gcol + 2]
                    b = sm[:, gcol + 2:gcol + 4]
                for c in range(EC):
                    # tb (bf16, feeds the next matmuls -> critical path) on DVE;
                    # tT (fp32 residual, consumed later) on gpsimd
                    nc.vector.tensor_scalar(out=tb_new[:, c, :], in0=c2[:, c, :],
                                            scalar1=g[:, c:c + 1], scalar2=b[:, c:c + 1],
                                            op0=Alu.mult, op1=Alu.add)
                    nc.gpsimd.tensor_scalar(out=t_new[:, c, :], in0=c2[:, c, :],
                                            scalar1=g[:, c:c + 1], scalar2=b[:, c:c + 1],
                                            op0=Alu.mult, op1=Alu.add)
                return t_new, tb_new

            def attention(q_sb, k_sb, v_sb, k_tiles, mask_sb, name, g_exp=G_EXP):
                """generic attention: q_sb/k_sb: [128, C, NQ/N] bf16 feature-major;
                v_sb: [128, ntile, H, 33]; returns attn [128, 2, 300] bf16"""
                attn = acts.tile([PC, EC, NQ], bf, tag=f"attn", name=f"attn{name}")
                nkt = len(k_tiles)
                for h in range(H):
                    po = 32 * (h % 4)
                    ci = h // 4
                    qh = q_sb[po:po + 32, ci, 0:NQ]
                    av = ps_pp.tile([PC, NQ], f32, tag="pp", name=f"av{name}h{h}")
                    g = 0
                    while g < nkt:
                        gsz = min(g_exp, nkt - g)
                        kg = k_tiles[g][1]  # uniform tile height in this group
                        assert all(k_tiles[g + j][1] == kg for j in range(gsz))
                        sc = ps_sc.tile([PC, G_EXP, 512], f32, tag="sc",
                                        name=f"sc{name}h{h}g{g}")
                        for j in range(gsz):
                            kt0, ksz = k_tiles[g + j]
                            nc.tensor.matmul(
                                out=sc[0:ksz, j, 0:NQ],
                                lhsT=k_sb[po:po + 32, ci, kt0:kt0 + ksz],
                                rhs=qh, start=True, stop=True,
                                tile_position=(po, 0))
                        p = probs.tile([PC, G_EXP, NQ], bf, tag="p",
                                       name=f"p{name}h{h}g{g}")
                        nc.scalar.activation(out=p[0:kg, 0:gsz, :],
                                             in_=sc[0:kg, 0:gsz, 0:NQ], func=Act.Exp)
                        if mask_sb is not None:
                            pm = probs.tile([PC, G_EXP, NQ], bf, tag="pm",
                                            name=f"pm{name}h{h}g{g}")
                            nc.vector.tensor_mul(out=pm[0:kg, 0:gsz, :],
                                                 in0=p[0:kg, 0:gsz, :],
                                                 in1=mask_sb[0:kg, g:g + gsz, :])
                        else:
                            pm = p
                        for j in range(gsz):
                            kt0, ksz = k_tiles[g + j]
                            ti = g + j
                            nc.tensor.matmul(
                                out=av[0:2 * D, 0:NQ],
                                lhsT=v_sb[0:ksz, ti, h, 0:2 * D],
                                rhs=pm[0:ksz, j, 0:NQ],
                                start=(ti == 0), stop=(ti == nkt - 1),
                                tile_position=(0, 0))
                        g += gsz
                    recip = stp.tile([32, NQ], f32, tag="recip", name=f"rc{name}h{h}", bufs=4)
                    nc.vector.reciprocal(out=recip, in_=av[D:2 * D, 0:NQ])
                    nc.vector.tensor_mul(out=attn[po:po + 32, ci, :],
                                         in0=av[0:32, 0:NQ], in1=recip)
                return attn

            def out_proj_residual(l, w_sb, attn, bcol, tT, name):
                r = acts.tile([PC, EC, NQ], f32, tag="r", name=f"r{name}", bufs=1)
                for co in range(EC):
                    po = ps_pp.tile([PC, NQ], f32, tag="pp", name=f"po{name}{co}")
                    for ci in range(EC):
                        nc.tensor.matmul(out=po, lhsT=w_sb[:, ci, PC * co:PC * (co + 1)],
                                         rhs=attn[:, ci, :],
                                         start=(ci == 0), stop=(ci == EC - 1))
                    nc.vector.scalar_tensor_tensor(
                        out=r[:, co, :], in0=po, scalar=sm[:, bcol + co:bcol + co + 1],
                        in1=tT[:, co, :], op0=Alu.add, op1=Alu.add)
                return r

            def emit_kproj(l, wck, sm):
                kT = acts.tile([PC, EC, NK], bass.mybir.dt.bfloat16, tag="kT",
                               name=f"kT{l}", bufs=2)
                for co in range(EC):
                    for nk in range(NKC):
                        pk = ps_pp.tile([PC, 512], f32, tag="pp", name=f"pk{l}_{co}_{nk}")
                        for ci in range(EC):
                            nc.tensor.matmul(out=pk,
                                             lhsT=wck[:, ci, PC * co:PC * (co + 1)],
                                             rhs=memT[:, ci, 512 * nk:512 * (nk + 1)],
                                             start=(ci == 0), stop=(ci == EC - 1))
                        nc.vector.tensor_scalar(
                            out=kT[:, co, 512 * nk:512 * (nk + 1)], in0=pk,
                            scalar1=sm[:, C_BK_CA + co:C_BK_CA + co + 1],
                            scalar2=None, op0=Alu.add)
                return kT

            kT_next = None
            wsm_next = None
            for l in range(nlayers):
                # ---- layer weight loads ----
                wqk = wp.tile([PC, EC, 2 * E], bass.mybir.dt.bfloat16, tag="wqk", name=f"wqk{l}")
                nc.sync.dma_start(out=wqk, in_=r2(d_wqk[l]))
                wsv = wp.tile([PC, EC, E], bass.mybir.dt.bfloat16, tag="wsv", name=f"wsv{l}")
                nc.sync.dma_start(out=wsv, in_=r2(d_wsv[l]))
                wso = wp.tile([PC, EC, E], bass.mybir.dt.bfloat16, tag="wso", name=f"wso{l}")
                nc.sync.dma_start(out=wso, in_=r2(d_wso[l]))
                wcq = wp.tile([PC, EC, E], bass.mybir.dt.bfloat16, tag="wcq", name=f"wcq{l}")
                nc.sync.dma_start(out=wcq, in_=r2(d_wcq[l]))
                if l == 0:
                    wck = wp.tile([PC, EC, E], bass.mybir.dt.bfloat16, tag="wck", name=f"wck{l}")
                    nc.sync.dma_start(out=wck, in_=r2(d_wck[l]))
                wcv = wp.tile([PC, EC, E], bass.mybir.dt.bfloat16, tag="wcv", name=f"wcv{l}")
                nc.sync.dma_start(out=wcv, in_=r2(d_wcv[l]))
                wco = wp.tile([PC, EC, E], bass.mybir.dt.bfloat16, tag="wco", name=f"wco{l}")
                nc.sync.dma_start(out=wco, in_=r2(d_wco[l]))
                w1 = wp.tile([PC, EC, F], bass.mybir.dt.bfloat16, tag="w1", name=f"w1_{l}", bufs=1)
                nc.sync.dma_start(out=w1, in_=r2(d_w1[l]))
                w2 = wp.tile([PC, FT, E], bass.mybir.dt.bfloat16, tag="w2", name=f"w2_{l}", bufs=1)
                nc.sync.dma_start(out=w2, in_=d_w2[l].rearrange("(c p) o -> p c o", p=PC))
                if l == 0:
                    sm = wp.tile([PC, NS], f32, tag="sm", name=f"sm{l}")
                    nc.sync.dma_start(out=sm, in_=d_sm[l])
                else:
                    sm = wsm_next
                vrow = wp.tile([PC, 2, E], f32, tag="vrow", name=f"vrow{l}")
                nc.sync.dma_start(out=vrow, in_=d_vrow[l])

                # ---- SA qkv projections ----
                qk_sa = acts.tile([PC, 4, NQ], bass.mybir.dt.bfloat16, tag="qk_sa", name=f"qk_sa{l}")
                if True:
                    for co in range(4):
                        po = ps_pp.tile([PC, NQ], f32, tag="pp", name=f"pqk{l}_{co}")
                        for ci in range(EC):
                            nc.tensor.matmul(out=po, lhsT=wqk[:, ci, PC * co:PC * (co + 1)],
                                             rhs=tb[:, ci, :],
                                             start=(ci == 0), stop=(ci == EC - 1))
                        # q gets the attention scale folded in
                        nc.vector.tensor_scalar(
                            out=qk_sa[:, co, :], in0=po,
                            scalar1=sm[:, C_BQK + co:C_BQK + co + 1],
                            scalar2=SCALE if co < 2 else 1.0,
                            op0=Alu.add, op1=Alu.mult)
                    for tt, (t0, tsz) in enumerate(TOK_TILES):
                        pv = ps_pp.tile([PC, E], f32, tag="pp", name=f"pvsa{l}_{tt}")
                        for ci in range(EC):
                            nc.tensor.matmul(out=pv[0:tsz, :],
                                             lhsT=tb[:, ci, t0:t0 + tsz],
                                             rhs=wsv[:, ci, :],
                                             start=(ci == 0), stop=(ci == EC - 1))
                        nc.vector.tensor_add(
                            out=vsa[0:tsz, tt, :, 0:D],
                            in0=_hd(pv[0:tsz, :]),
                            in1=_hd(vrow[0:tsz, 0, :]))

                # ---- SA attention ----
                attn = attention(qk_sa[:, 0:2, :], qk_sa[:, 2:4, :],
                                 vsa, TOK_TILES, None, f"sa{l}", g_exp=3)
                # CA v-projection hoisted here: depends only on memT/wcv, and
                # the WAR on vca (prev layer's CA attention) is already clear.


                # ---- SA out proj + LN1 ----
                r = out_proj_residual(l, wso, attn, C_BO_SA, tT, f"so{l}")
                tT, tb = layernorm(l, r, C_LN, f"ln1_{l}")

                # ---- CA projections ----
                q_ca = acts.tile([PC, EC, NQ], bass.mybir.dt.bfloat16, tag="q_ca", name=f"q_ca{l}")
                kT = kT_next if kT_next is not None else emit_kproj(l, wck, sm)
                kT_next = None
                if True:
                    for co in range(EC):
                        po = ps_pp.tile([PC, NQ], f32, tag="pp", name=f"pq_ca{l}_{co}")
                        for ci in range(EC):
                            nc.tensor.matmul(out=po, lhsT=wcq[:, ci, PC * co:PC * (co + 1)],
                                             rhs=tb[:, ci, :],
                                             start=(ci == 0), stop=(ci == EC - 1))
                        nc.vector.tensor_scalar(
                            out=q_ca[:, co, :], in0=po,
                            scalar1=sm[:, C_BQ_CA + co:C_BQ_CA + co + 1],
                            scalar2=SCALE, op0=Alu.add, op1=Alu.mult)
                    for tt in range(KT_CA):
                        pv = ps_pp.tile([PC, E], f32, tag="pp", name=f"pvca{l}_{tt}")
                        for ci in range(EC):
                            nc.tensor.matmul(out=pv,
                                             lhsT=memT[:, ci, PC * tt:PC * (tt + 1)],
                                             rhs=wcv[:, ci, :],
                                             start=(ci == 0), stop=(ci == EC - 1))
                        nc.vector.tensor_add(
                            out=vca[:, tt, :, 0:D],
                            in0=_hd(pv),
                            in1=_hd(vrow[:, 1, :]))

                # ---- CA attention ----
                ca_tiles = [(PC * i, PC) for i in range(KT_CA)]
                attn = attention(q_ca, kT, vca, ca_tiles, maskT, f"ca{l}")
                if l + 1 < nlayers:
                    wck_n = wp.tile([PC, EC, E], bass.mybir.dt.bfloat16, tag="wck", name=f"wck{l + 1}")
                    nc.sync.dma_start(out=wck_n, in_=r2(d_wck[l + 1]))
                    sm_n = wp.tile([PC, NS], f32, tag="sm", name=f"sm{l + 1}")
                    nc.sync.dma_start(out=sm_n, in_=d_sm[l + 1])
                    kT_next = emit_kproj(l + 1, wck_n, sm_n)
                    wsm_next = sm_n

                # ---- CA out proj + LN2 ----
                r = out_proj_residual(l, wco, attn, C_BO_CA, tT, f"co{l}")
                tT, tb = layernorm(l, r, C_LN + 4, f"ln2_{l}")

                # ---- FFN ----
                hT = acts.tile([PC, FT, NQ], bass.mybir.dt.bfloat16, tag="hT", name=f"hT{l}", bufs=1)
                if True:
                    for ft in range(FT):
                        pf = ps_pp.tile([PC, NQ], f32, tag="pp", name=f"pf1_{l}_{ft}")
                        for ci in range(EC):
                            nc.tensor.matmul(out=pf, lhsT=w1[:, ci, PC * ft:PC * (ft + 1)],
                                             rhs=tb[:, ci, :],
                                             start=(ci == 0), stop=(ci == EC - 1))
                        if ft % 2 == 0:
                            nc.scalar.activation(out=hT[:, ft, :], in_=pf, func=Act.Relu,
                                                 bias=sm[:, C_B1 + ft:C_B1 + ft + 1])
                        else:
                            nc.vector.tensor_scalar(
                                out=hT[:, ft, :], in0=pf,
                                scalar1=sm[:, C_B1 + ft:C_B1 + ft + 1], scalar2=0.0,
                                op0=Alu.add, op1=Alu.max)
                    r = acts.tile([PC, EC, NQ], f32, tag="r", name=f"rf{l}", bufs=1)
                    for co in range(EC):
                        p2 = ps_pp.tile([PC, NQ], f32, tag="pp", name=f"pf2_{l}_{co}")
                        for fc in range(FT):
                            nc.tensor.matmul(out=p2, lhsT=w2[:, fc, PC * co:PC * (co + 1)],
                                             rhs=hT[:, fc, :],
                                             start=(fc == 0), stop=(fc == FT - 1))
                        nc.vector.scalar_tensor_tensor(
                            out=r[:, co, :], in0=p2,
                            scalar=sm[:, C_B2 + co:C_B2 + co + 1],
                            in1=tT[:, co, :], op0=Alu.add, op1=Alu.add)
                tT, tb = layernorm(l, r, C_LN + 8, f"ln3_{l}")

            # ---- final LN + store ----
            outT, _ = layernorm(None, tT, None, "lnf")
            nc.sync.dma_start(out=r2(d_out), in_=outT)

    nc.compile()
    return nc


def _pack_inputs(inputs, nlayers=L):
    """Host-side layout prep: transpose / cast / pack. Returns per-core in_maps."""
    bf = BF16
    smalls = np.zeros((nlayers, PC, NS), np.float32)
    vrow = np.zeros((nlayers, PC, 2, E), np.float32)
    for l in range(nlayers):
        def put(col, vec):
            n = vec.shape[0] // PC
            smalls[l, :, col:col + n] = vec.reshape(n, PC).T
        put(C_BQK, np.asarray(inputs["sa_bqkv"][l][:2 * E], np.float32))
        put(C_BO_SA, np.asarray(inputs["sa_bo"][l], np.float32))
        put(C_BQ_CA, np.asarray(inputs["ca_bq"][l], np.float32))
        put(C_BK_CA, np.asarray(inputs["ca_bk"][l], np.float32))
        put(C_BO_CA, np.asarray(inputs["ca_bo"][l], np.float32))
        put(C_B1, np.asarray(inputs["f_b1"][l], np.float32))
        put(C_B2, np.asarray(inputs["f_b2"][l], np.float32))
        put(C_LN, np.asarray(inputs["ln1g"][l], np.float32))
        put(C_LN + 2, np.asarray(inputs["ln1b"][l], np.float32))
        put(C_LN + 4, np.asarray(inputs["ln2g"][l], np.float32))
        put(C_LN + 6, np.asarray(inputs["ln2b"][l], np.float32))
        put(C_LN + 8, np.asarray(inputs["ln3g"][l], np.float32))
        put(C_LN + 10, np.asarray(inputs["ln3b"][l], np.float32))
        vrow[l, :, 0, :] = np.asarray(inputs["sa_bqkv"][l][2 * E:], np.float32)[None, :]
        vrow[l, :, 1, :] = np.asarray(inputs["ca_bv"][l], np.float32)[None, :]
    finals = np.zeros((PC, 4), np.float32)
    finals[:, 0:2] = np.asarray(inputs["lnfg"], np.float32).reshape(2, PC).T
    finals[:, 2:4] = np.asarray(inputs["lnfb"], np.float32).reshape(2, PC).T

    def T(x):
        return np.ascontiguousarray(np.swapaxes(np.asarray(x), -1, -2))

    shared = {
        "w_sa_qk": T(inputs["sa_wqkv"][:nlayers, :2 * E]).astype(bf),
        "w_sa_v": T(inputs["sa_wqkv"][:nlayers, 2 * E:]).astype(bf),
        "w_sa_o": T(inputs["sa_wo"][:nlayers]).astype(bf),
        "w_ca_q": T(inputs["ca_wq"][:nlayers]).astype(bf),
        "w_ca_k": T(inputs["ca_wk"][:nlayers]).astype(bf),
        "w_ca_v": T(inputs["ca_wv"][:nlayers]).astype(bf),
        "w_ca_o": T(inputs["ca_wo"][:nlayers]).astype(bf),
        "w_f1": T(inputs["f_w1"][:nlayers]).astype(bf),
        "w_f2": T(inputs["f_w2"][:nlayers]).astype(bf),
        "smalls": smalls,
        "vrow": vrow,
        "finals": finals,
    }
    in_maps = []
    for b in range(B):
        m = dict(shared)
        m["tT"] = T(inputs["tgt"][b]).astype(np.float32)
        m["memT"] = T(inputs["memory"][b]).astype(bf)
        m["maskT"] = T(inputs["geometry_mask"][b]).astype(bf)
        in_maps.append(m)
    return in_maps


_CACHE = {}


def kernel(run_opts=None, **inputs):
    nlayers = L
    if "nc" not in _CACHE:
        _CACHE["nc"] = build_nc(nlayers)
    nc = _CACHE["nc"]
    in_maps = _pack_inputs(inputs, nlayers)
    res = bass_utils.run_bass_kernel_spmd(
        nc, in_maps, core_ids=list(range(B)), **(run_opts or {}))
    _CACHE["last_result"] = res
    out = np.stack([np.asarray(r["outT"]).T for r in res.results])
    return np.ascontiguousarray(out.astype(np.float32))
